# revision 15
# baseline (speedup 1.0000x reference)
"""Causal self-attention (QKV projection + softmax(QK^T/sqrt(N)) @ V) on 8 TRN2
NeuronCores.

Sharding: core c = 2*b + j handles batch element b (of 4) and half the query
rows (two 512-row blocks from opposite ends of the causal triangle).

The kernel never materializes K or V. Both big projections are reassociated so
the per-core work is proportional to the core's OWN 1024 queries instead of
the full 2048-key sequence (which is duplicated across the core pair):

  scores = (ctx Wk + bk)(ctx Wq + bq)^T
         = ctx (Wk Wq^T) ctx^T + a_k + (q-terms that cancel in softmax)
    -> host folds Wkq^T = (Wq Wk^T)/sqrt(N) (weight-only), device computes
       U = Wkq^T ctx_q^T  [D x 512 per query block], then S^T = ctx^T.T-tiles
       @ U per key tile; a_k = ctx (Wk bq)/sqrt(N) is a host matvec shipped as
       a per-key-tile activation bias for the Exp.
  out   = P (ctx Wv + bv) = (P^T ctx) Wv + bv   (sum P = 1 after normalize)
    -> device computes op^T = ctx_rows^T-tiles @ P per d-tile, then
       out = op Wv / den + bv.

Per-core big (512-free) matmuls: 128 (U) + 192 (S) + 192 (P^T ctx) + 128 (Wv)
= 640, vs 1024 for the direct QKV formulation. All operands bf16 (f32 PSUM);
simulated end-to-end rel err ~4.5e-3 vs the 2e-2 gate.

Uniform-SPMD trick: each core processes the four 512-token chunks in a
per-core PERMUTED order (j=0: [0,1,2,3]; j=1: [1,0,3,2]) so the chunk at
schedule position 0 is always the core's low query block and position 3 its
high block; per-core causal masks built from shipped position vectors make the
uniform schedule correct. Everything is SBUF-resident (~18MB): the attention
phase does zero HBM traffic except output writes.
"""

import math
from contextlib import ExitStack

import numpy as np

import concourse.bass as bass
import concourse.mybir as mybir
import concourse.tile as tile
from concourse.bass_utils import run_bass_kernel_spmd
from concourse.tile_rust import add_dep_helper

P = 128
CH = 512  # ctx chunk columns == query block rows == max f32 PSUM free dim


def _fix_matmul_waits(nc):
    """Walrus codegen has a small per-instruction sync-wait slot budget (one
    for a self-loading matmul's LDWEIGHTS half, similar for ACT etc). Move
    extra waits onto NoOps inserted just before the instruction on the same
    engine — per-engine program order (and thus semantics) is unchanged."""
    skip = (mybir.InstEventSemaphore, mybir.InstNoOp,
            mybir.InstUnconditionalBranch, mybir.InstCall)
    for func in nc.m.functions:
        for bb in func.blocks:
            il = bb.instructions
            new = []
            changed = False
            for inst in il:
                si = getattr(inst, "sync_info", None)
                if (si and si.on_wait and len(si.on_wait) > 1
                        and not isinstance(inst, skip)):
                    waits = list(si.on_wait)
                    for wi, w in enumerate(waits[:-1]):
                        nop = mybir.InstNoOp(
                            name=f"{inst.name}-wfix{wi}", engine=inst.engine,
                            sync_info=mybir.SyncInfo(on_wait=[w], on_update=[]),
                            text_hint="waitfix")
                        new.append(nop)
                    inst.sync_info = mybir.SyncInfo(
                        on_wait=[waits[-1]], on_update=list(si.on_update or []))
                    changed = True
                new.append(inst)
            if changed:
                bb.instructions = new


def build(N=2048, D=1024, has_bias=True, fix_waits=True, **bass_kwargs):
    NT = N // P          # schedule k-tiles
    DN = D // P          # 128-blocks of the model dim
    NCH = N // CH        # ctx chunks == query blocks per batch
    QBLK = CH
    QT = QBLK // P
    QTOT = 2 * QBLK
    BF = mybir.dt.bfloat16
    F32 = mybir.dt.float32
    AF = mybir.ActivationFunctionType
    OP = mybir.AluOpType

    nc = bass.Bass(**bass_kwargs)

    ctxT = nc.declare_dram_parameter("ctxT", [D, N], BF, isOutput=False)
    ctxR = nc.declare_dram_parameter("ctxR", [N, D], BF, isOutput=False)
    wkqT = nc.declare_dram_parameter("wkqT", [D, D], BF, isOutput=False)
    wvd = nc.declare_dram_parameter("wvd", [D, D], BF, isOutput=False)
    qpos = nc.declare_dram_parameter("qpos", [P, QTOT], F32, isOutput=False)
    kpos = nc.declare_dram_parameter("kpos", [P, NT], F32, isOutput=False)
    abk = nc.declare_dram_parameter("abk", [P, NT], F32, isOutput=False)
    bvb = nc.declare_dram_parameter("bvb", [P, D], F32, isOutput=False)
    onesd = nc.declare_dram_parameter("onesd", [P, 8], BF, isOutput=False)
    out_ext = nc.declare_dram_parameter("out", [QTOT, D], BF, isOutput=True)

    with ExitStack() as ctx:
        tc = ctx.enter_context(tile.TileContext(nc))
        const = ctx.enter_context(tc.tile_pool(name="const", bufs=1))
        persist = ctx.enter_context(tc.tile_pool(name="persist", bufs=1))
        mpool = ctx.enter_context(tc.tile_pool(name="mp", bufs=3))
        rpool = ctx.enter_context(tc.tile_pool(name="rp", bufs=1))
        opool = ctx.enter_context(tc.tile_pool(name="op", bufs=3))
        pp = ctx.enter_context(tc.tile_pool(name="pp", bufs=4, space="PSUM"))
        dpp = ctx.enter_context(tc.tile_pool(name="dpp", bufs=1, space="PSUM"))

        # SBUF stores (bf16):
        #   wkq_sb: lhsT tile (t_out, c_contract) at col (t*DN+c)*P      (2MB)
        #   cts[pos]: ctx^T chunk, col d*CH + token                  (4x 1MB)
        #   ctr_sb: ctx rows, col kt*D + d                               (4MB)
        #   wv_sb: d-major, col d*D + e                                  (2MB)
        #   u_store: col t*QTOT + qb*QBLK + q                            (2MB)
        #   e_all: col kt*QBLK + q                                       (2MB)
        #   op_sb: col d*QBLK + q (per-qb scratch)                       (1MB)
        wkq_sb = persist.tile([P, D * DN], BF, name="wkq")
        wv_sb = persist.tile([P, D * DN], BF, name="wv")
        cts = [persist.tile([P, DN * CH], BF, name=f"ct{c}") for c in range(NCH)]
        ctr_sb = persist.tile([P, NT * D], BF, name="ctr")
        u_store = persist.tile([P, DN * QTOT], BF, name="u_store")
        e_all = persist.tile([P, NT * QBLK], BF, name="e_all")
        op_sb = persist.tile([P, DN * QBLK], BF, name="op_sb")

        qpos_sb = const.tile([P, QTOT], F32)
        kpos_sb = const.tile([P, NT], F32)
        ab_sb = const.tile([P, NT], F32)
        bv_sb = const.tile([P, D], F32)
        ones_sb = const.tile([P, 8], BF)

        # ---- DMA stream, in exact consumption order ----
        # Startup-critical loads (wkq block 0 + ctx chunk 0 + rest of wkq) go
        # on the GpSimd queue, whose engine preamble finishes ~3us before
        # Sync's; the bulk Sync stream is gated behind the first U matmul so
        # the critical 3MB gets the full HBM rate.
        def ld_colblk(eng, dst, src, t):  # e/t-column block t of a [D, D] matrix
            return eng.dma_start(
                out=dst[:, t * D:(t + 1) * D].rearrange("p (c x) -> p c x", x=P),
                in_=src[:, t * P:(t + 1) * P].rearrange("(c p) x -> p c x", p=P))

        def ld_ctx(eng, pos):
            return eng.dma_start(
                out=cts[pos].rearrange("p (d c) -> p d c", c=CH),
                in_=ctxT[:, pos * CH:(pos + 1) * CH].rearrange("(d p) c -> p d c", p=P))

        ld_colblk(nc.gpsimd, wkq_sb, wkqT, 0)
        ld_ctx(nc.gpsimd, 0)
        for t in range(1, DN):
            ld_colblk(nc.gpsimd, wkq_sb, wkqT, t)
        sync_dmas = []
        sync_dmas.append(ld_ctx(nc.sync, 3))            # U(qb1) source
        if has_bias:
            sync_dmas.append(nc.sync.dma_start(out=ab_sb, in_=abk[:, :]))
        sync_dmas.append(ld_ctx(nc.sync, 1))
        sync_dmas.append(nc.sync.dma_start(out=qpos_sb, in_=qpos[:, :]))
        sync_dmas.append(nc.sync.dma_start(out=kpos_sb, in_=kpos[:, :]))
        sync_dmas.append(nc.sync.dma_start(out=ones_sb, in_=onesd[:, :]))
        sync_dmas.append(ld_ctx(nc.sync, 2))
        sync_dmas.append(nc.sync.dma_start(  # ctx rows, schedule-k-tile-permuted
            out=ctr_sb.rearrange("p (t d) -> p t d", d=D),
            in_=ctxR.rearrange("(t p) d -> p t d", p=P)))
        for d in range(DN):           # wv d-major
            sync_dmas.append(nc.sync.dma_start(out=wv_sb[:, d * D:(d + 1) * D],
                                               in_=wvd[d * P:(d + 1) * P, :]))
        if has_bias:
            sync_dmas.append(nc.sync.dma_start(out=bv_sb, in_=bvb[:, :]))

        # ---- U = Wkq^T @ ctx_q^T for both query blocks ----
        first_mm = None
        for qb, pos in ((0, 0), (1, NCH - 1)):
            for t in range(DN):
                psu = pp.tile([P, CH], F32, tag="big", name="psu")
                for c in range(DN):
                    mm = nc.tensor.matmul(
                        psu, lhsT=wkq_sb[:, (t * DN + c) * P:(t * DN + c + 1) * P],
                        rhs=cts[pos][:, c * CH:(c + 1) * CH],
                        start=(c == 0), stop=(c == DN - 1))
                    if first_mm is None:
                        first_mm = mm
                nc.scalar.activation(
                    u_store[:, t * QTOT + qb * QBLK:t * QTOT + (qb + 1) * QBLK],
                    psu, AF.Copy)
        for dma in sync_dmas:
            add_dep_helper(dma.ins, first_mm.ins, sync=True, reason="dma staging")

        # ---- attention ----
        for qb in range(2):
            KT = NT // 2 if qb == 0 else NT
            # scores + exp + mask
            for k in range(KT):
                pos, loc = divmod(k, CH // P)
                pss = pp.tile([P, QBLK], F32, tag="big", name="pss")
                for d in range(DN):
                    nc.tensor.matmul(
                        pss, lhsT=cts[pos][:, d * CH + loc * P:d * CH + (loc + 1) * P],
                        rhs=u_store[:, d * QTOT + qb * QBLK:d * QTOT + (qb + 1) * QBLK],
                        start=(d == 0), stop=(d == DN - 1))
                esl = e_all[:, k * QBLK:(k + 1) * QBLK]
                if has_bias:
                    nc.scalar.activation(esl, pss, AF.Exp, bias=ab_sb[:, k:k + 1])
                else:
                    nc.scalar.activation(esl, pss, AF.Exp)
                if qb == 0 or k >= NT // 2:
                    m = mpool.tile([P, QBLK], BF, tag="m", name="m")
                    nc.vector.tensor_scalar(m, qpos_sb[:, qb * QBLK:(qb + 1) * QBLK],
                                            kpos_sb[:, k:k + 1], None, OP.is_ge)
                    nc.vector.tensor_tensor(esl, esl, m, OP.mult)
            # denominators (psd[qt] accumulates over k) + reciprocals
            psd = [dpp.tile([P, 8], F32, tag=f"den{qt}", name="psd") for qt in range(QT)]
            for qt in range(QT):
                for k in range(KT):
                    nc.tensor.matmul(psd[qt],
                                     lhsT=e_all[:, k * QBLK + qt * P:k * QBLK + (qt + 1) * P],
                                     rhs=ones_sb, start=(k == 0), stop=(k == KT - 1))
            recs = []
            for qt in range(QT):
                rec = rpool.tile([P, 1], F32, tag=f"rec{qt}", name="rec")
                nc.vector.reciprocal(rec, psd[qt][:, 0:1])
                recs.append(rec)
            # op^T = ctx_rows^T-tiles @ P  (d-tile at a time, 1 PSUM bank each)
            for d in range(DN):
                ppv = pp.tile([P, QBLK], F32, tag="big", name="ppv")
                for k in range(KT):
                    nc.tensor.matmul(
                        ppv, lhsT=ctr_sb[:, k * D + d * P:k * D + (d + 1) * P],
                        rhs=e_all[:, k * QBLK:(k + 1) * QBLK],
                        start=(k == 0), stop=(k == KT - 1))
                nc.scalar.activation(op_sb[:, d * QBLK:(d + 1) * QBLK], ppv, AF.Copy)
            # out = op Wv / den + bv
            for qt in range(QT):
                for ei in range(D // CH):
                    psf = pp.tile([P, CH], F32, tag="big", name="psf")
                    for d in range(DN):
                        nc.tensor.matmul(
                            psf, lhsT=op_sb[:, d * QBLK + qt * P:d * QBLK + (qt + 1) * P],
                            rhs=wv_sb[:, d * D + ei * CH:d * D + (ei + 1) * CH],
                            start=(d == 0), stop=(d == DN - 1))
                    ot = opool.tile([P, CH], BF, tag="o", name="ot")
                    if has_bias:
                        nc.vector.tensor_scalar_mul(ot, psf, recs[qt])
                        nc.vector.tensor_tensor(ot, ot, bv_sb[:, ei * CH:(ei + 1) * CH], OP.add)
                    else:
                        # normalize on Scalar: out = psf * (1/den), same engine
                        # as the out DMA so the tail has no cross-engine hop
                        nc.scalar.activation(ot, psf, AF.Copy, scale=recs[qt])
                    nc.scalar.dma_start(
                        out=out_ext[qb * QBLK + qt * P:qb * QBLK + (qt + 1) * P,
                                    ei * CH:(ei + 1) * CH],
                        in_=ot)
    if fix_waits:
        _fix_matmul_waits(nc)
    return nc


def _chunk_order(j):
    # schedule position 0 = low query block, position 3 = high query block.
    return [0, 1, 2, 3] if j == 0 else [1, 0, 3, 2]


def make_in_maps(context, W_qkv, b_qkv, n_cores=8):
    import ml_dtypes
    bf16 = ml_dtypes.bfloat16
    context = np.ascontiguousarray(np.asarray(context, np.float32))
    W_qkv = np.asarray(W_qkv, np.float32)
    b_qkv = np.asarray(b_qkv, np.float32)
    B, N, D = context.shape
    NT = N // P
    QTOT = 2 * CH
    SCALE = 1.0 / math.sqrt(N)
    Wq, Wk, Wv = W_qkv[:, :D], W_qkv[:, D:2 * D], W_qkv[:, 2 * D:]
    bq, bk, bv = b_qkv[:D], b_qkv[D:2 * D], b_qkv[2 * D:]
    wkqT = np.ascontiguousarray(((Wq @ Wk.T) * SCALE).astype(bf16))
    wvd = np.ascontiguousarray(Wv.astype(bf16))
    bvb = np.ascontiguousarray(np.broadcast_to(bv, (P, D)).astype(np.float32))
    wkbq = (Wk @ bq) * SCALE  # [D]; a_k = ctx_k . wkbq (k-dependent exp bias)
    in_maps = []
    for c in range(n_cores):
        b, j = divmod(c, 2)
        order = _chunk_order(j)
        ctx_b = context[b]
        ctx_bT = ctx_b.T.astype(bf16)
        ctxT = np.ascontiguousarray(np.concatenate(
            [ctx_bT[:, o * CH:(o + 1) * CH] for o in order], axis=1))
        ctxR = np.ascontiguousarray(np.concatenate(
            [ctx_b[o * CH:(o + 1) * CH] for o in order], axis=0).astype(bf16))
        qpos_row = np.concatenate([
            np.arange(order[0] * CH, (order[0] + 1) * CH),
            np.arange(order[3] * CH, (order[3] + 1) * CH)]).astype(np.float32)
        qpos_b = np.ascontiguousarray(np.broadcast_to(qpos_row, (P, QTOT)))
        kpos = np.empty((P, NT), np.float32)
        abk = np.empty((P, NT), np.float32)
        a_full = ctx_b @ wkbq  # [N]
        for t in range(NT):
            keys = order[t // 4] * CH + (t % 4) * P + np.arange(P)
            kpos[:, t] = keys
            abk[:, t] = a_full[keys]
        in_maps.append({
            "ctxT": ctxT, "ctxR": ctxR, "wkqT": wkqT, "wvd": wvd,
            "qpos": qpos_b, "kpos": np.ascontiguousarray(kpos),
            "abk": np.ascontiguousarray(abk), "bvb": bvb,
            "onesd": np.ones((P, 8), bf16),
        })
    return in_maps


def assemble(results, B, N, D):
    out = np.zeros((B, N, D), np.float32)
    for c, res in enumerate(results):
        b, j = divmod(c, 2)
        order = _chunk_order(j)
        o = np.asarray(res["out"], np.float32)
        out[b, order[0] * CH:(order[0] + 1) * CH] = o[:CH]
        out[b, order[3] * CH:(order[3] + 1) * CH] = o[CH:]
    return out


def run(inputs, trace=False, **spmd_kwargs):
    context = np.asarray(inputs["context"])
    B, N, D = context.shape
    has_bias = bool(np.any(np.asarray(inputs["b_qkv"])))
    nc = build(N, D, has_bias=has_bias)
    in_maps = make_in_maps(context, inputs["W_qkv"], inputs["b_qkv"], n_cores=8)
    res = run_bass_kernel_spmd(nc, in_maps, core_ids=list(range(8)), trace=trace, **spmd_kwargs)
    out = assemble(res.results, B, N, D)
    return out, res


def kernel(context, W_qkv, b_qkv):
    out, _ = run({"context": context, "W_qkv": W_qkv, "b_qkv": b_qkv})
    return out


# revision 17
# speedup vs baseline: 1.0113x; 1.0113x over previous
"""Causal self-attention (QKV projection + softmax(QK^T/sqrt(N)) @ V) on 8 TRN2
NeuronCores.

Sharding: core c = 2*b + j handles batch element b (of 4) and half the query
rows (two 512-row blocks from opposite ends of the causal triangle).

The kernel never materializes K or V. Both big projections are reassociated so
the per-core work is proportional to the core's OWN 1024 queries instead of
the full 2048-key sequence (which is duplicated across the core pair):

  scores = (ctx Wk + bk)(ctx Wq + bq)^T
         = ctx (Wk Wq^T) ctx^T + a_k + (q-terms that cancel in softmax)
    -> host folds Wkq^T = (Wq Wk^T)/sqrt(N) (weight-only), device computes
       U = Wkq^T ctx_q^T  [D x 512 per query block], then S^T = ctx^T.T-tiles
       @ U per key tile; a_k = ctx (Wk bq)/sqrt(N) is a host matvec shipped as
       a per-key-tile activation bias for the Exp.
  out   = P (ctx Wv + bv) = (P^T ctx) Wv + bv   (sum P = 1 after normalize)
    -> device computes op^T = ctx_rows^T-tiles @ P per d-tile, then
       out = op Wv / den + bv.

Per-core big (512-free) matmuls: 128 (U) + 192 (S) + 192 (P^T ctx) + 128 (Wv)
= 640, vs 1024 for the direct QKV formulation. All operands bf16 (f32 PSUM);
simulated end-to-end rel err ~4.5e-3 vs the 2e-2 gate.

Uniform-SPMD trick: each core processes the four 512-token chunks in a
per-core PERMUTED order (j=0: [0,1,2,3]; j=1: [1,0,3,2]) so the chunk at
schedule position 0 is always the core's low query block and position 3 its
high block; per-core causal masks built from shipped position vectors make the
uniform schedule correct. Everything is SBUF-resident (~18MB): the attention
phase does zero HBM traffic except output writes.
"""

import math
from contextlib import ExitStack

import numpy as np

import concourse.bass as bass
import concourse.mybir as mybir
import concourse.tile as tile
from concourse.bass_utils import run_bass_kernel_spmd
from concourse.tile_rust import add_dep_helper

P = 128
CH = 512  # ctx chunk columns == query block rows == max f32 PSUM free dim


def _fix_matmul_waits(nc):
    """Walrus codegen has a small per-instruction sync-wait slot budget (one
    for a self-loading matmul's LDWEIGHTS half, similar for ACT etc). Move
    extra waits onto NoOps inserted just before the instruction on the same
    engine — per-engine program order (and thus semantics) is unchanged."""
    skip = (mybir.InstEventSemaphore, mybir.InstNoOp,
            mybir.InstUnconditionalBranch, mybir.InstCall)
    for func in nc.m.functions:
        for bb in func.blocks:
            il = bb.instructions
            new = []
            changed = False
            for inst in il:
                si = getattr(inst, "sync_info", None)
                if (si and si.on_wait and len(si.on_wait) > 1
                        and not isinstance(inst, skip)):
                    waits = list(si.on_wait)
                    for wi, w in enumerate(waits[:-1]):
                        nop = mybir.InstNoOp(
                            name=f"{inst.name}-wfix{wi}", engine=inst.engine,
                            sync_info=mybir.SyncInfo(on_wait=[w], on_update=[]),
                            text_hint="waitfix")
                        new.append(nop)
                    inst.sync_info = mybir.SyncInfo(
                        on_wait=[waits[-1]], on_update=list(si.on_update or []))
                    changed = True
                new.append(inst)
            if changed:
                bb.instructions = new


def build(N=2048, D=1024, has_bias=True, fix_waits=True, **bass_kwargs):
    NT = N // P          # schedule k-tiles
    DN = D // P          # 128-blocks of the model dim
    NCH = N // CH        # ctx chunks == query blocks per batch
    QBLK = CH
    QT = QBLK // P
    QTOT = 2 * QBLK
    BF = mybir.dt.bfloat16
    F32 = mybir.dt.float32
    AF = mybir.ActivationFunctionType
    OP = mybir.AluOpType

    nc = bass.Bass(**bass_kwargs)

    ctxT = nc.declare_dram_parameter("ctxT", [D, N], BF, isOutput=False)
    ctxR = nc.declare_dram_parameter("ctxR", [N, D], BF, isOutput=False)
    wkqT = nc.declare_dram_parameter("wkqT", [D, D], BF, isOutput=False)
    wvd = nc.declare_dram_parameter("wvd", [D, D], BF, isOutput=False)
    qpos = nc.declare_dram_parameter("qpos", [P, QTOT], F32, isOutput=False)
    kpos = nc.declare_dram_parameter("kpos", [P, NT], F32, isOutput=False)
    abk = nc.declare_dram_parameter("abk", [P, NT], F32, isOutput=False)
    bvb = nc.declare_dram_parameter("bvb", [P, D], F32, isOutput=False)
    onesd = nc.declare_dram_parameter("onesd", [P, 8], BF, isOutput=False)
    out_ext = nc.declare_dram_parameter("out", [QTOT, D], BF, isOutput=True)

    with ExitStack() as ctx:
        tc = ctx.enter_context(tile.TileContext(nc))
        const = ctx.enter_context(tc.tile_pool(name="const", bufs=1))
        persist = ctx.enter_context(tc.tile_pool(name="persist", bufs=1))
        mpool = ctx.enter_context(tc.tile_pool(name="mp", bufs=3))
        rpool = ctx.enter_context(tc.tile_pool(name="rp", bufs=1))
        opool = ctx.enter_context(tc.tile_pool(name="op", bufs=3))
        pp = ctx.enter_context(tc.tile_pool(name="pp", bufs=4, space="PSUM"))
        dpp = ctx.enter_context(tc.tile_pool(name="dpp", bufs=1, space="PSUM"))

        # SBUF stores (bf16):
        #   wkq_sb: lhsT tile (t_out, c_contract) at col (t*DN+c)*P      (2MB)
        #   cts[pos]: ctx^T chunk, col d*CH + token                  (4x 1MB)
        #   ctr_sb: ctx rows, col kt*D + d                               (4MB)
        #   wv_sb: d-major, col d*D + e                                  (2MB)
        #   u_store: col t*QTOT + qb*QBLK + q                            (2MB)
        #   e_all: col kt*QBLK + q                                       (2MB)
        #   op_sb: col d*QBLK + q (per-qb scratch)                       (1MB)
        wkq_sb = persist.tile([P, D * DN], BF, name="wkq")
        wv_sb = persist.tile([P, D * DN], BF, name="wv")
        cts = [persist.tile([P, DN * CH], BF, name=f"ct{c}") for c in range(NCH)]
        ctr_sb = persist.tile([P, NT * D], BF, name="ctr")
        u_store = persist.tile([P, DN * QTOT], BF, name="u_store")
        e_all = persist.tile([P, NT * QBLK], BF, name="e_all")
        op_sb = persist.tile([P, DN * QBLK], BF, name="op_sb")

        qpos_sb = const.tile([P, QTOT], F32)
        kpos_sb = const.tile([P, NT], F32)
        ab_sb = const.tile([P, NT], F32)
        bv_sb = const.tile([P, D], F32)
        ones_sb = const.tile([P, 8], BF)

        # ---- DMA stream, in exact consumption order (one sync HW ring) ----
        # ctx chunk 0 is split per d-block so the first U matmul can start
        # after ~0.4MB instead of the whole 1.25MB critical prefix.
        def ld_colblk(eng, dst, src, t):  # e/t-column block t of a [D, D] matrix
            return eng.dma_start(
                out=dst[:, t * D:(t + 1) * D].rearrange("p (c x) -> p c x", x=P),
                in_=src[:, t * P:(t + 1) * P].rearrange("(c p) x -> p c x", p=P))

        def ld_ctx(eng, pos):
            return eng.dma_start(
                out=cts[pos].rearrange("p (d c) -> p d c", c=CH),
                in_=ctxT[:, pos * CH:(pos + 1) * CH].rearrange("(d p) c -> p d c", p=P))

        ld_colblk(nc.sync, wkq_sb, wkqT, 0)
        for dblk in range(DN):  # ctx chunk 0, one d-block at a time
            nc.sync.dma_start(
                out=cts[0][:, dblk * CH:(dblk + 1) * CH],
                in_=ctxT[dblk * P:(dblk + 1) * P, 0:CH])
        for t in range(1, DN):
            ld_colblk(nc.sync, wkq_sb, wkqT, t)
        sync_dmas = []
        sync_dmas.append(ld_ctx(nc.sync, 3))            # U(qb1) source
        if has_bias:
            sync_dmas.append(nc.sync.dma_start(out=ab_sb, in_=abk[:, :]))
        sync_dmas.append(ld_ctx(nc.sync, 1))
        sync_dmas.append(nc.sync.dma_start(out=qpos_sb, in_=qpos[:, :]))
        sync_dmas.append(nc.sync.dma_start(out=kpos_sb, in_=kpos[:, :]))
        sync_dmas.append(nc.sync.dma_start(out=ones_sb, in_=onesd[:, :]))
        sync_dmas.append(ld_ctx(nc.sync, 2))
        sync_dmas.append(nc.sync.dma_start(  # ctx rows, schedule-k-tile-permuted
            out=ctr_sb.rearrange("p (t d) -> p t d", d=D),
            in_=ctxR.rearrange("(t p) d -> p t d", p=P)))
        for d in range(DN):           # wv d-major
            sync_dmas.append(nc.sync.dma_start(out=wv_sb[:, d * D:(d + 1) * D],
                                               in_=wvd[d * P:(d + 1) * P, :]))
        if has_bias:
            sync_dmas.append(nc.sync.dma_start(out=bv_sb, in_=bvb[:, :]))

        # ---- U = Wkq^T @ ctx_q^T for both query blocks ----
        # (qb0's U runs at 256-free in q-halves: probes the bf16 small-free
        # matmul rate, and lets compute start after ctx0's first d-blocks)
        first_mm = None
        for qb, pos in ((0, 0), (1, NCH - 1)):
            for t in range(DN):
                psu = pp.tile([P, CH], F32, tag="big", name="psu")
                halves = (0, CH // 2) if qb == 0 else (0,)
                hsz = CH // len(halves)
                for h in halves:
                    for c in range(DN):
                        mm = nc.tensor.matmul(
                            psu[:, h:h + hsz],
                            lhsT=wkq_sb[:, (t * DN + c) * P:(t * DN + c + 1) * P],
                            rhs=cts[pos][:, c * CH + h:c * CH + h + hsz],
                            start=(c == 0), stop=(c == DN - 1))
                        if first_mm is None:
                            first_mm = mm
                nc.scalar.activation(
                    u_store[:, t * QTOT + qb * QBLK:t * QTOT + (qb + 1) * QBLK],
                    psu, AF.Copy)
        for dma in sync_dmas:
            add_dep_helper(dma.ins, first_mm.ins, sync=True, reason="dma staging")

        # ---- attention ----
        for qb in range(2):
            KT = NT // 2 if qb == 0 else NT
            # scores + exp + mask
            for k in range(KT):
                pos, loc = divmod(k, CH // P)
                pss = pp.tile([P, QBLK], F32, tag="big", name="pss")
                for d in range(DN):
                    nc.tensor.matmul(
                        pss, lhsT=cts[pos][:, d * CH + loc * P:d * CH + (loc + 1) * P],
                        rhs=u_store[:, d * QTOT + qb * QBLK:d * QTOT + (qb + 1) * QBLK],
                        start=(d == 0), stop=(d == DN - 1))
                esl = e_all[:, k * QBLK:(k + 1) * QBLK]
                if has_bias:
                    nc.scalar.activation(esl, pss, AF.Exp, bias=ab_sb[:, k:k + 1])
                else:
                    nc.scalar.activation(esl, pss, AF.Exp)
                if qb == 0 or k >= NT // 2:
                    m = mpool.tile([P, QBLK], BF, tag="m", name="m")
                    nc.vector.tensor_scalar(m, qpos_sb[:, qb * QBLK:(qb + 1) * QBLK],
                                            kpos_sb[:, k:k + 1], None, OP.is_ge)
                    nc.vector.tensor_tensor(esl, esl, m, OP.mult)
            # denominators (psd[qt] accumulates over k) + reciprocals
            psd = [dpp.tile([P, 8], F32, tag=f"den{qt}", name="psd") for qt in range(QT)]
            for qt in range(QT):
                for k in range(KT):
                    nc.tensor.matmul(psd[qt],
                                     lhsT=e_all[:, k * QBLK + qt * P:k * QBLK + (qt + 1) * P],
                                     rhs=ones_sb, start=(k == 0), stop=(k == KT - 1))
            recs = []
            for qt in range(QT):
                rec = rpool.tile([P, 1], F32, tag=f"rec{qt}", name="rec")
                nc.vector.reciprocal(rec, psd[qt][:, 0:1])
                recs.append(rec)
            # op^T = ctx_rows^T-tiles @ P  (d-tile at a time, 1 PSUM bank each)
            for d in range(DN):
                ppv = pp.tile([P, QBLK], F32, tag="big", name="ppv")
                for k in range(KT):
                    nc.tensor.matmul(
                        ppv, lhsT=ctr_sb[:, k * D + d * P:k * D + (d + 1) * P],
                        rhs=e_all[:, k * QBLK:(k + 1) * QBLK],
                        start=(k == 0), stop=(k == KT - 1))
                nc.scalar.activation(op_sb[:, d * QBLK:(d + 1) * QBLK], ppv, AF.Copy)
            # out = op Wv / den + bv
            for qt in range(QT):
                for ei in range(D // CH):
                    psf = pp.tile([P, CH], F32, tag="big", name="psf")
                    for d in range(DN):
                        nc.tensor.matmul(
                            psf, lhsT=op_sb[:, d * QBLK + qt * P:d * QBLK + (qt + 1) * P],
                            rhs=wv_sb[:, d * D + ei * CH:d * D + (ei + 1) * CH],
                            start=(d == 0), stop=(d == DN - 1))
                    ot = opool.tile([P, CH], BF, tag="o", name="ot")
                    if has_bias:
                        nc.vector.tensor_scalar_mul(ot, psf, recs[qt])
                        nc.vector.tensor_tensor(ot, ot, bv_sb[:, ei * CH:(ei + 1) * CH], OP.add)
                    else:
                        # normalize on Scalar: out = psf * (1/den), same engine
                        # as the out DMA so the tail has no cross-engine hop
                        nc.scalar.activation(ot, psf, AF.Copy, scale=recs[qt])
                    nc.scalar.dma_start(
                        out=out_ext[qb * QBLK + qt * P:qb * QBLK + (qt + 1) * P,
                                    ei * CH:(ei + 1) * CH],
                        in_=ot)
    if fix_waits:
        _fix_matmul_waits(nc)
    return nc


def _chunk_order(j):
    # schedule position 0 = low query block, position 3 = high query block.
    return [0, 1, 2, 3] if j == 0 else [1, 0, 3, 2]


def make_in_maps(context, W_qkv, b_qkv, n_cores=8):
    import ml_dtypes
    bf16 = ml_dtypes.bfloat16
    context = np.ascontiguousarray(np.asarray(context, np.float32))
    W_qkv = np.asarray(W_qkv, np.float32)
    b_qkv = np.asarray(b_qkv, np.float32)
    B, N, D = context.shape
    NT = N // P
    QTOT = 2 * CH
    SCALE = 1.0 / math.sqrt(N)
    Wq, Wk, Wv = W_qkv[:, :D], W_qkv[:, D:2 * D], W_qkv[:, 2 * D:]
    bq, bk, bv = b_qkv[:D], b_qkv[D:2 * D], b_qkv[2 * D:]
    wkqT = np.ascontiguousarray(((Wq @ Wk.T) * SCALE).astype(bf16))
    wvd = np.ascontiguousarray(Wv.astype(bf16))
    bvb = np.ascontiguousarray(np.broadcast_to(bv, (P, D)).astype(np.float32))
    wkbq = (Wk @ bq) * SCALE  # [D]; a_k = ctx_k . wkbq (k-dependent exp bias)
    in_maps = []
    for c in range(n_cores):
        b, j = divmod(c, 2)
        order = _chunk_order(j)
        ctx_b = context[b]
        ctx_bT = ctx_b.T.astype(bf16)
        ctxT = np.ascontiguousarray(np.concatenate(
            [ctx_bT[:, o * CH:(o + 1) * CH] for o in order], axis=1))
        ctxR = np.ascontiguousarray(np.concatenate(
            [ctx_b[o * CH:(o + 1) * CH] for o in order], axis=0).astype(bf16))
        qpos_row = np.concatenate([
            np.arange(order[0] * CH, (order[0] + 1) * CH),
            np.arange(order[3] * CH, (order[3] + 1) * CH)]).astype(np.float32)
        qpos_b = np.ascontiguousarray(np.broadcast_to(qpos_row, (P, QTOT)))
        kpos = np.empty((P, NT), np.float32)
        abk = np.empty((P, NT), np.float32)
        a_full = ctx_b @ wkbq  # [N]
        for t in range(NT):
            keys = order[t // 4] * CH + (t % 4) * P + np.arange(P)
            kpos[:, t] = keys
            abk[:, t] = a_full[keys]
        in_maps.append({
            "ctxT": ctxT, "ctxR": ctxR, "wkqT": wkqT, "wvd": wvd,
            "qpos": qpos_b, "kpos": np.ascontiguousarray(kpos),
            "abk": np.ascontiguousarray(abk), "bvb": bvb,
            "onesd": np.ones((P, 8), bf16),
        })
    return in_maps


def assemble(results, B, N, D):
    out = np.zeros((B, N, D), np.float32)
    for c, res in enumerate(results):
        b, j = divmod(c, 2)
        order = _chunk_order(j)
        o = np.asarray(res["out"], np.float32)
        out[b, order[0] * CH:(order[0] + 1) * CH] = o[:CH]
        out[b, order[3] * CH:(order[3] + 1) * CH] = o[CH:]
    return out


def run(inputs, trace=False, **spmd_kwargs):
    context = np.asarray(inputs["context"])
    B, N, D = context.shape
    has_bias = bool(np.any(np.asarray(inputs["b_qkv"])))
    nc = build(N, D, has_bias=has_bias)
    in_maps = make_in_maps(context, inputs["W_qkv"], inputs["b_qkv"], n_cores=8)
    res = run_bass_kernel_spmd(nc, in_maps, core_ids=list(range(8)), trace=trace, **spmd_kwargs)
    out = assemble(res.results, B, N, D)
    return out, res


def kernel(context, W_qkv, b_qkv):
    out, _ = run({"context": context, "W_qkv": W_qkv, "b_qkv": b_qkv})
    return out
